# revision 10
# baseline (speedup 1.0000x reference)
"""Causal self-attention (S=2048, B=2, D=768, H=12) on 8 TRN2 NeuronCores.

Sharding: batch*heads across cores. Core c handles batch b = c//4 and the
3 heads hs = (c%4)*3 .. hs+2. Each core computes Q/K/V projections for its
heads, causal softmax(QK^T/sqrt(hd)) @ V, and its partial contribution to
the output projection y_part = att_cat @ wc_slice^T. The host gathers by
summing the 4 per-batch partials and adding the output bias.

Numerics: all matmuls bf16 with fp32 PSUM accumulation. The causal mask is
applied inside the scores accumulation chain (maskneg^T @ ident adds -1e9
above the diagonal), the 1/sqrt(64) score scale is folded into the EXP
activation's scale parameter, and the softmax denominator falls out of a
ones-column appended to V. K's projection bias is dropped: it shifts every
key's score for a given query equally, which softmax cancels.

Schedule: per-core wqk columns are grouped per head ([q_h | k_h] in each
128-col block) so head h's scores can start right after projection block h
drains - the scalar engine (EXP, the longest-pole engine with the PE) spins
up ~8us in. The main loop staggers heads by one round (scores(h, kb=r-h))
and lags AV/normalize by AVLAG rounds and transpose/out-proj/output-DMA one
round further, so the in-order tensor queue always has ready filler work
between EXP-paced score chunks. attT0 transposes ride the idle DMA XBAR.
"""

import numpy as np
import ml_dtypes

import concourse.bass as bass
import concourse.mybir as mybir
import concourse.tile as tile
from concourse import bacc
from concourse.bass_utils import run_bass_kernel_spmd

S = 2048  # sequence length
B = 2     # batch
D = 768   # model dim
H = 12    # heads
HD = 64   # head dim
NCORES = 8
HPC = 3   # heads per core
DC = HPC * HD          # 192: per-core head dims
VW = HPC * (HD + 1)    # 195: V columns incl per-head ones column
NQB = S // 128         # 16 query/key blocks
AVLAG = 5              # rounds between scores(kb) and AV/normalize(qi)
OPLAG = 6              # rounds between scores(kb) and out-proj(qj)
F32 = mybir.dt.float32
BF16 = mybir.dt.bfloat16
BF = ml_dtypes.bfloat16

TRACE = False          # set by test harness for profiled runs
LAST_RESULT = None     # BassKernelResults of the most recent run

_prog_cache = {}


def _build_program():
    nc = bacc.Bacc()
    AF = mybir.ActivationFunctionType

    xtp = nc.declare_dram_parameter("xtp", [128, 6, S], BF16, isOutput=False)
    wqkp = nc.declare_dram_parameter("wqkp", [128, 6, 2 * DC], BF16, isOutput=False)
    wvp = nc.declare_dram_parameter("wvp", [128, 6, VW], BF16, isOutput=False)
    wvrow = nc.declare_dram_parameter("wvrow", [1, VW], BF16, isOutput=False)
    bq = nc.declare_dram_parameter("bq", [HPC * 128, 1], F32, isOutput=False)
    ga = nc.declare_dram_parameter("ga", [128, D], BF16, isOutput=False)
    gb = nc.declare_dram_parameter("gb", [128, D], BF16, isOutput=False)
    y = nc.declare_dram_parameter("y", [S, D], BF16, isOutput=True)

    with tile.TileContext(nc) as tc:
        with (
            tc.tile_pool(name="const", bufs=1) as constp,
            tc.tile_pool(name="acts", bufs=1) as actsp,
            tc.tile_pool(name="roll", bufs=2) as rollp,
            tc.tile_pool(name="small", bufs=4) as smallp,
            tc.tile_pool(name="mm", bufs=6, space="PSUM") as mmp,
            tc.tile_pool(name="po", bufs=2, space="PSUM") as pop,
        ):
            # ---- constants / weights ----
            # mask01[k, q] = 1 iff k <= q: zeroes pt entries above the
            # diagonal after EXP (keys later than the query)
            from concourse.masks import make_upper_triangular
            mask01 = constp.tile([128, 128], BF16, tag="mask", name="mask01")
            make_upper_triangular(nc, mask01[:], val=1.0, diag=True)

            xt_sb = constp.tile([128, 6, S], BF16, tag="xtp", name="xt_sb")
            wqk_sb = constp.tile([128, 6, 2 * DC], BF16, tag="wqkp", name="wqk_sb")
            wv_sb = constp.tile([128, 6, VW], BF16, tag="wvp", name="wv_sb")
            for k in range(6):
                nc.sync.dma_start(wqk_sb[:, k, :], wqkp[:, k, :])
                nc.sync.dma_start(xt_sb[:, k, 0:1024], xtp[:, k, 0:1024])
                nc.sync.dma_start(xt_sb[:, k, 1024:S], xtp[:, k, 1024:S])
                nc.sync.dma_start(wv_sb[:, k, :], wvp[:, k, :])
            wvr_sb = constp.tile([1, VW], BF16, tag="wvrow", name="wvr_sb")
            nc.sync.dma_start(wvr_sb[:], wvrow[:])
            wvr_bc = constp.tile([128, VW], BF16, tag="wvrbc", name="wvr_bc")
            nc.gpsimd.partition_broadcast(wvr_bc[:], wvr_sb[:])
            bq_sb = []
            for h in range(HPC):
                t = constp.tile([128, 1], F32, tag=f"bq{h}", name=f"bq{h}")
                nc.sync.dma_start(t[:], bq[h * 128:(h + 1) * 128, :])
                bq_sb.append(t)
            ga_sb = constp.tile([128, D], BF16, tag="ga", name="ga_sb")
            nc.sync.dma_start(ga_sb[:], ga[:])
            gb_sb = constp.tile([128, D], BF16, tag="gb", name="gb_sb")
            nc.sync.dma_start(gb_sb[:], gb[:])

            # scores contraction is zero-padded to K=128: matmuls with a
            # sub-128-partition rhs stream ~1.6x slower on hw, so qt rows
            # 64:128 carry (unused, finite) K values and kt rows 64:128 are
            # zeroed once, making the padded dot products vanish.
            qt = [constp.tile([128, S], BF16, tag=f"qt{h}", name=f"qt{h}")
                  for h in range(HPC)]
            kt = [constp.tile([128, S], BF16, tag=f"kt{h}", name=f"kt{h}")
                  for h in range(HPC)]
            for h in range(HPC):
                nc.vector.memset(kt[h][64:128, :], 0.0)
            v_sb = [actsp.tile([128, VW], BF16, tag=f"v{kb}", name=f"v{kb}")
                    for kb in range(NQB)]
            # pt[h][kb][:, j] = exp(s[kb*128 + :, kb*128 + j]/8); exact causal
            # width, live until the last AV chain reads it.
            pt = [[actsp.tile([128, S - kb * 128], BF16, tag=f"pt{h}_{kb}",
                              name=f"pt{h}_{kb}") for kb in range(NQB)]
                  for h in range(HPC)]

            def proj_qk(h):
                # wqk col block h = [q_h (64) | k_h (64)]
                for n in range(4):
                    ps = mmp.tile([128, 512], F32, tag="mm", name="psqk")
                    for k in range(6):
                        nc.tensor.matmul(
                            ps[:], wqk_sb[:, k, h * 128:(h + 1) * 128],
                            xt_sb[:, k, n * 512:(n + 1) * 512],
                            start=(k == 0), stop=(k == 5))
                    cols = slice(n * 512, (n + 1) * 512)
                    nc.vector.tensor_scalar_add(
                        qt[h][:, cols], ps[:], bq_sb[h][:])
                    nc.vector.tensor_copy(kt[h][0:64, cols], ps[64:128, :])

            def proj_v(kb):
                ps = mmp.tile([128, 512], F32, tag="mm", name="psv")
                reg = ps[:, 0:VW]
                for k in range(6):
                    nc.tensor.matmul(
                        reg, xt_sb[:, k, kb * 128:(kb + 1) * 128],
                        wv_sb[:, k, :], start=(k == 0), stop=(k == 5))
                nc.vector.tensor_add(v_sb[kb][:], reg, wvr_bc[:])

            def scores(h, kb):
                W = S - kb * 128
                for ci in range((W + 511) // 512):
                    n = min(512, W - ci * 512)
                    qs = kb * 128 + ci * 512
                    ps = mmp.tile([128, 512], F32, tag="mm", name="pss")
                    nc.tensor.matmul(
                        ps[:, 0:n], kt[h][:, kb * 128:(kb + 1) * 128],
                        qt[h][:, qs:qs + n], start=True, stop=True)
                    nc.scalar.activation(
                        pt[h][kb][:, ci * 512:ci * 512 + n],
                        ps[:, 0:n], AF.Exp, scale=0.125)
                    if ci == 0:
                        nc.vector.tensor_mul(
                            pt[h][kb][:, 0:128], pt[h][kb][:, 0:128], mask01[:])

            def av_block(qi):
                # one [128, 195] psum: 3 heads side by side; denom in col 64+65h
                po = pop.tile([128, VW], F32, tag="po", name="po")
                for h in range(HPC):
                    for kb2 in range(qi + 1):
                        nc.tensor.matmul(
                            po[:, h * 65:h * 65 + 65],
                            pt[h][kb2][:, (qi - kb2) * 128:(qi - kb2 + 1) * 128],
                            v_sb[kb2][:, h * 65:h * 65 + 65],
                            start=(kb2 == 0), stop=(kb2 == qi))
                rr = smallp.tile([128, HPC], F32, tag="r", name="rr")
                nc.vector.reciprocal(rr[:], po[:, 64::65])
                att3 = rollp.tile([128, DC], BF16, tag="att3", name="att3")
                for h in range(HPC):
                    nc.vector.tensor_scalar_mul(
                        att3[:, h * 64:(h + 1) * 64],
                        po[:, h * 65:h * 65 + 64], rr[:, h:h + 1])
                # attT via the DMA XBAR, as overlapping 128-dim halves
                # (dims 0:128 and 64:192); ga's zeroed rows 64:128 keep the
                # out-proj from double-counting dims 64:128.
                a0 = rollp.tile([128, 128], BF16, tag="attT0", name="a0")
                nc.sync.dma_start_transpose(a0[:], att3[:, 0:128])
                a1 = rollp.tile([128, 128], BF16, tag="attT1", name="a1")
                nc.sync.dma_start_transpose(a1[:], att3[:, 64:192])
                return a0, a1

            def outproj(qj, a0, a1):
                ys = rollp.tile([128, D], BF16, tag="ys", name="ys")
                for (n0, nsz) in ((0, 512), (512, 256)):
                    ps = mmp.tile([128, 512], F32, tag="mm", name="psy")
                    nc.tensor.matmul(ps[:, 0:nsz], a0[:], ga_sb[:, n0:n0 + nsz],
                                     start=True, stop=False)
                    nc.tensor.matmul(ps[:, 0:nsz], a1[:], gb_sb[:, n0:n0 + nsz],
                                     start=False, stop=True)
                    nc.vector.tensor_copy(ys[:, n0:n0 + nsz], ps[:, 0:nsz])
                nc.sync.dma_start(y[qj * 128:(qj + 1) * 128, 0:512], ys[:, 0:512])
                nc.sync.dma_start(y[qj * 128:(qj + 1) * 128, 512:D], ys[:, 512:D])

            # ---- emission schedule ----
            proj_qk(0)
            proj_qk(1)
            attT = {}
            for r in range(NQB + 2 + OPLAG):
                # lagged AV / normalize / transpose first: always-ready filler
                # for the in-order tensor queue while EXP drains score chunks
                qi = r - AVLAG
                if 0 <= qi < NQB:
                    attT[qi] = av_block(qi)
                qj = r - OPLAG
                if 0 <= qj < NQB:
                    a0, a1 = attT.pop(qj)
                    outproj(qj, a0, a1)
                for h in range(HPC):
                    kb = r - h
                    if 0 <= kb < NQB:
                        scores(h, kb)
                if r == 0:
                    proj_qk(2)
                elif r in (1, 2):
                    for kb in range(8 * (r - 1), 8 * r):
                        proj_v(kb)

    nc.finalize()
    return nc


def _pack_contraction(a):
    """[768, N] -> [128, 6, N]: row j -> (partition j%128, chunk j//128)."""
    n = a.shape[1]
    return np.ascontiguousarray(
        a.reshape(6, 128, n).transpose(1, 0, 2)).astype(BF)


def _prep_inputs(x, wq, bq, wk, bk, wv, bv, wc, bc):
    """Per-core input maps, all host-side slicing/transposition."""
    in_maps = []
    for c in range(NCORES):
        b = c // 4
        r0 = (c % 4) * HPC * HD
        rows = slice(r0, r0 + DC)
        xtb = np.ascontiguousarray(x[:, b, :].T)        # [768, 2048]
        wqk = np.empty((D, 2 * DC), np.float32)         # [q_h|k_h] per block
        wva = np.zeros((D, VW), np.float32)
        wvr = np.zeros((1, VW), np.float32)
        for j in range(HPC):
            hr = slice(r0 + j * HD, r0 + (j + 1) * HD)
            wqk[:, j * 128:j * 128 + 64] = wq[hr].T
            wqk[:, j * 128 + 64:j * 128 + 128] = wk[hr].T
            wva[:, j * 65:j * 65 + HD] = wv[hr].T
            wvr[0, j * 65:j * 65 + HD] = bv[hr]
            wvr[0, j * 65 + HD] = 1.0
        gm = np.ascontiguousarray(wc[:, rows].T).astype(np.float32)  # [192, 768]
        gam = np.zeros((128, D), np.float32)
        gam[0:64] = gm[0:64]
        bqp = np.zeros((HPC * 128, 1), np.float32)
        for j in range(HPC):
            bqp[j * 128:j * 128 + 64, 0] = bq[r0 + j * HD:r0 + (j + 1) * HD]
        in_maps.append({
            "xtp": _pack_contraction(xtb),
            "wqkp": _pack_contraction(wqk),
            "wvp": _pack_contraction(wva),
            "wvrow": wvr.astype(BF),
            "bq": bqp,
            "ga": gam.astype(BF),
            "gb": np.ascontiguousarray(gm[64:192]).astype(BF),
        })
    return in_maps


def kernel(**inputs):
    global LAST_RESULT
    if "prog" not in _prog_cache:
        _prog_cache["prog"] = _build_program()
    nc = _prog_cache["prog"]

    args = {k: np.asarray(inputs[k], np.float32)
            for k in ("x", "wq", "bq", "wk", "bk", "wv", "bv", "wc", "bc")}
    in_maps = _prep_inputs(**args)
    res = run_bass_kernel_spmd(nc, in_maps, core_ids=list(range(NCORES)),
                               trace=TRACE)
    LAST_RESULT = res

    out = np.empty((S, B, D), np.float32)
    for b in range(B):
        acc = res.results[4 * b]["y"].astype(np.float32)
        for c in range(4 * b + 1, 4 * b + 4):
            acc = acc + res.results[c]["y"].astype(np.float32)
        out[:, b, :] = acc + args["bc"][None, :]
    return out
